# revision 3
# baseline (speedup 1.0000x reference)
"""TRN2 Bass kernel: out = inp @ weights + bias.

Shapes (hardcoded): inp [16384, 4096] f32, weights [4096, 8192] f32,
bias [8192] f32 -> out [16384, 8192] f32.

Strategy:
  - 8 NeuronCores as a 4 (batch) x 2 (contraction K) grid.
    Core c = (bi, ki) computes the partial GEMM
      part[bi,ki] = inpT[ki*2048:(ki+1)*2048, bi*4096:(bi+1)*4096].T
                    @ weights[ki*2048:(ki+1)*2048, :]
    i.e. per-core M=4096, K=2048, N=8192.
    The host sums the two K-partials and adds bias while assembling the
    full output (exact fp32 adds).
  - Matmuls run in float32r (TF32-style single-pass mode): full PE speed
    for free-dim >= 256, ~1.5e-4 rel error at this K, vs 4x slower fp32.
  - The K-split doubles the SBUF-cacheable M-panel, which halves the
    re-streaming of the weight operand: per-core HBM traffic ~410 MB,
    hidden under ~1.75 ms of PE work.
"""
import sys

sys.path.insert(0, "/opt/trn_rl_repo")

import numpy as np

B, F, C = 16384, 4096, 8192
P = 128
NB, NK = 4, 2               # grid: batch-splits x K-splits
MB_CORE = B // NB           # 4096  per-core M
KB_CORE = F // NK           # 2048  per-core K
KS = KB_CORE // P           # 16    K subtiles of 128
M_BLK = 1024                # SBUF-cached M panel width
N_TILE = 512                # PSUM bank width (fp32)
M_BLKS = MB_CORE // M_BLK   # 4
M_SUBS = M_BLK // P         # 8
N_TILES = C // N_TILE       # 16

_compiled = None
_last_in_maps = None


def _build():
    import concourse.mybir as mybir
    import concourse.tile as tile
    from concourse import bacc
    from concourse.bass_interp import get_hw_module

    nc = bacc.Bacc("TRN2", target_bir_lowering=False, debug=False, num_devices=8)

    x_dram = nc.dram_tensor("x", [KB_CORE, MB_CORE], mybir.dt.float32r, kind="ExternalInput")
    w_dram = nc.dram_tensor("w", [KB_CORE, C], mybir.dt.float32r, kind="ExternalInput")
    out_dram = nc.dram_tensor("out", [MB_CORE, C], mybir.dt.float32, kind="ExternalOutput")

    x_ap = x_dram.ap().rearrange("(ko p) m -> p ko m", p=P)   # [128, KS, 4096]
    w_ap = w_dram.ap().rearrange("(ko p) n -> p ko n", p=P)   # [128, KS, 8192]
    out_ap = out_dram.ap()

    with tile.TileContext(nc) as tc:
        with tc.tile_pool(name="kxm", bufs=2) as kxm_pool, \
             tc.tile_pool(name="kxn", bufs=2) as kxn_pool, \
             tc.tile_pool(name="outp", bufs=4) as out_pool, \
             tc.tile_pool(name="ps", bufs=4, space="PSUM") as psum_pool:
            for mb in range(M_BLKS):
                kxm = kxm_pool.tile([P, KS, M_BLK], mybir.dt.float32r, tag="kxm")
                nc.sync.dma_start(kxm[:], x_ap[:, :, mb * M_BLK:(mb + 1) * M_BLK])
                for nt in range(N_TILES):
                    kxn = kxn_pool.tile([P, KS, N_TILE], mybir.dt.float32r, tag="kxn")
                    nc.sync.dma_start(kxn[:], w_ap[:, :, nt * N_TILE:(nt + 1) * N_TILE])
                    for ms in range(M_SUBS):
                        psum = psum_pool.tile([P, N_TILE], mybir.dt.float32, tag="ps")
                        for ks in range(KS):
                            nc.tensor.matmul(
                                psum[:],
                                kxm[:, ks, ms * P:(ms + 1) * P],
                                kxn[:, ks, :],
                                start=(ks == 0),
                                stop=(ks == KS - 1),
                            )
                        ot = out_pool.tile([P, N_TILE], mybir.dt.float32, tag="ot")
                        nc.vector.tensor_copy(ot[:], psum[:])
                        nc.sync.dma_start(
                            out_ap[mb * M_BLK + ms * P: mb * M_BLK + (ms + 1) * P,
                                   nt * N_TILE:(nt + 1) * N_TILE],
                            ot[:],
                        )

    nc.compile()
    nc.m = get_hw_module(nc.m)
    return nc


def _transpose(a: np.ndarray) -> np.ndarray:
    """Fast-ish single-core host transpose of a 2D fp32 array."""
    try:
        import torch

        return torch.from_numpy(a).t().contiguous().numpy()
    except ImportError:
        pass
    r, c = a.shape
    bs = 128
    out = np.empty((c, r), np.float32)
    v = a.reshape(r // bs, bs, c // bs, bs)
    o = out.reshape(c // bs, bs, r // bs, bs)
    np.copyto(o, v.transpose(2, 3, 0, 1))
    return out


def kernel(inp: np.ndarray, weights: np.ndarray, bias: np.ndarray) -> np.ndarray:
    global _compiled
    from concourse import bass_utils

    if _compiled is None:
        _compiled = _build()
    nc = _compiled

    inp = np.ascontiguousarray(inp, dtype=np.float32)
    weights = np.ascontiguousarray(weights, dtype=np.float32)
    inpT = _transpose(inp)  # [F, B]

    in_maps = []
    for bi in range(NB):
        for ki in range(NK):
            x_c = np.ascontiguousarray(
                inpT[ki * KB_CORE:(ki + 1) * KB_CORE, bi * MB_CORE:(bi + 1) * MB_CORE]
            )
            w_c = weights[ki * KB_CORE:(ki + 1) * KB_CORE, :]
            in_maps.append({"x": x_c, "w": w_c})

    global _last_in_maps
    _last_in_maps = in_maps
    res = bass_utils.run_bass_kernel_spmd(nc, in_maps, list(range(NB * NK)))

    out = np.empty((B, C), np.float32)
    bias32 = bias.astype(np.float32, copy=False)
    for bi in range(NB):
        blk = out[bi * MB_CORE:(bi + 1) * MB_CORE]
        np.add(res.results[bi * NK]["out"], res.results[bi * NK + 1]["out"], out=blk)
        blk += bias32[None, :]
    return out


# revision 6
# speedup vs baseline: 2.4960x; 2.4960x over previous
"""TRN2 Bass kernel: out = inp @ weights + bias.

Shapes (hardcoded): inp [16384, 4096] f32, weights [4096, 8192] f32,
bias [8192] f32 -> out [16384, 8192] f32.

Strategy:
  - 8 NeuronCores as a 4 (batch) x 2 (contraction K) grid.
    Core c = (bi, ki) computes the partial GEMM
      part[bi,ki] = inpT[ki*2048:(ki+1)*2048, bi*4096:(bi+1)*4096].T
                    @ weights[ki*2048:(ki+1)*2048, :]
    i.e. per-core M=4096, K=2048, N=8192.
    The host sums the two K-partials and adds bias while assembling the
    full output (exact fp32 adds).
  - Matmuls run in float32r (TF32-style single-pass mode): full PE speed
    for free-dim >= 256, ~1.5e-4 rel error at this K, vs 4x slower fp32.
  - The K-split doubles the SBUF-cacheable M-panel, which halves the
    re-streaming of the weight operand: per-core HBM traffic ~410 MB,
    hidden under ~1.75 ms of PE work.
"""
import sys

sys.path.insert(0, "/opt/trn_rl_repo")

import numpy as np

B, F, C = 16384, 4096, 8192
P = 128
NB, NK = 4, 2               # grid: batch-splits x K-splits
MB_CORE = B // NB           # 4096  per-core M
KB_CORE = F // NK           # 2048  per-core K
KS = KB_CORE // P           # 16    K subtiles of 128
M_BLK = 1024                # SBUF-cached M panel width
N_TILE = 512                # PSUM bank width (fp32)
M_BLKS = MB_CORE // M_BLK   # 4
M_SUBS = M_BLK // P         # 8
N_TILES = C // N_TILE       # 16

_compiled = None
_last_in_maps = None


def _build(m_blks=M_BLKS, compile_hw=True):
    import concourse.mybir as mybir
    import concourse.tile as tile
    from concourse import bacc
    from concourse.bass_interp import get_hw_module

    nc = bacc.Bacc("TRN2", target_bir_lowering=False, debug=False, num_devices=8)

    x_dram = nc.dram_tensor("x", [KB_CORE, MB_CORE], mybir.dt.float32r, kind="ExternalInput")
    w_dram = nc.dram_tensor("w", [KB_CORE, C], mybir.dt.float32r, kind="ExternalInput")
    out_dram = nc.dram_tensor("out", [MB_CORE, C], mybir.dt.float32, kind="ExternalOutput")

    x_ap = x_dram.ap().rearrange("(ko p) m -> p ko m", p=P)   # [128, KS, 4096]
    w_ap = w_dram.ap().rearrange("(ko p) n -> p ko n", p=P)   # [128, KS, 8192]
    out_ap = out_dram.ap()

    with tile.TileContext(nc) as tc:
        with tc.tile_pool(name="kxm", bufs=2) as kxm_pool, \
             tc.tile_pool(name="kxn", bufs=2) as kxn_pool, \
             tc.tile_pool(name="outp", bufs=4) as out_pool, \
             tc.tile_pool(name="ps", bufs=4, space="PSUM") as psum_pool:
            for mb in range(m_blks):
                kxm = kxm_pool.tile([P, KS, M_BLK], mybir.dt.float32r, tag="kxm")
                nc.sync.dma_start(kxm[:], x_ap[:, :, mb * M_BLK:(mb + 1) * M_BLK])
                for nt in range(N_TILES):
                    kxn = kxn_pool.tile([P, KS, N_TILE], mybir.dt.float32r, tag="kxn")
                    nc.sync.dma_start(kxn[:], w_ap[:, :, nt * N_TILE:(nt + 1) * N_TILE])
                    for ms in range(M_SUBS):
                        psum = psum_pool.tile([P, N_TILE], mybir.dt.float32, tag="ps")
                        for ks in range(KS):
                            nc.tensor.matmul(
                                psum[:],
                                kxm[:, ks, ms * P:(ms + 1) * P],
                                kxn[:, ks, :],
                                start=(ks == 0),
                                stop=(ks == KS - 1),
                            )
                        ot = out_pool.tile([P, N_TILE], mybir.dt.float32, tag="ot")
                        nc.vector.tensor_copy(ot[:], psum[:])
                        nc.sync.dma_start(
                            out_ap[mb * M_BLK + ms * P: mb * M_BLK + (ms + 1) * P,
                                   nt * N_TILE:(nt + 1) * N_TILE],
                            ot[:],
                        )

    nc.compile()
    if compile_hw:
        nc.m = get_hw_module(nc.m)
    return nc


def _transpose(a: np.ndarray) -> np.ndarray:
    """Fast-ish single-core host transpose of a 2D fp32 array."""
    try:
        import torch

        return torch.from_numpy(a).t().contiguous().numpy()
    except ImportError:
        pass
    r, c = a.shape
    bs = 128
    out = np.empty((c, r), np.float32)
    v = a.reshape(r // bs, bs, c // bs, bs)
    o = out.reshape(c // bs, bs, r // bs, bs)
    np.copyto(o, v.transpose(2, 3, 0, 1))
    return out


def kernel(inp: np.ndarray, weights: np.ndarray, bias: np.ndarray) -> np.ndarray:
    global _compiled
    from concourse import bass_utils

    if _compiled is None:
        _compiled = _build()
    nc = _compiled

    inp = np.ascontiguousarray(inp, dtype=np.float32)
    weights = np.ascontiguousarray(weights, dtype=np.float32)
    inpT = _transpose(inp)  # [F, B]

    in_maps = []
    for bi in range(NB):
        for ki in range(NK):
            x_c = np.ascontiguousarray(
                inpT[ki * KB_CORE:(ki + 1) * KB_CORE, bi * MB_CORE:(bi + 1) * MB_CORE]
            )
            w_c = weights[ki * KB_CORE:(ki + 1) * KB_CORE, :]
            in_maps.append({"x": x_c, "w": w_c})

    global _last_in_maps
    _last_in_maps = in_maps
    res = bass_utils.run_bass_kernel_spmd(nc, in_maps, list(range(NB * NK)))

    out = np.empty((B, C), np.float32)
    bias32 = bias.astype(np.float32, copy=False)
    for bi in range(NB):
        blk = out[bi * MB_CORE:(bi + 1) * MB_CORE]
        np.add(res.results[bi * NK]["out"], res.results[bi * NK + 1]["out"], out=blk)
        blk += bias32[None, :]
    return out
